# Initial kernel scaffold
#
"""BiLSTM kernel for 8 Trainium2 NeuronCores.

Problem: B=64, T=1024, I=512, H=768, O=512 BiLSTM (gates f,i,g,o).

Sharding: core c in [0..7]: direction d = c//4 (0=fwd, 1=bwd over reversed
time), batch quarter q = c%4 (rows 16q..16q+16). Zero cross-core comms.

Per-core recurrence design (per time step, B_local=16):
  - z (16, 3072) = h @ Wh.T + x_t @ Wx.T + b computed as 4 concurrent
    column-group matmul streams (tile_position=(0, 32*g)): group g computes
    the H-slice [192g:192(g+1)) of every gate. PSUM chunk A = [i | g] gates,
    chunk B = [f | o] gates, each (128p, 384f) (partition window 32g+[0:16)
    = batch for group g).
  - Stationary operands are h^T tiles (128, 16) taken from hT, which is
    produced each step by a single DVE 32x32 block transpose of the
    group-layout h (128, 192). The block transpose leaves H-dims in a
    permuted order across (k-tile, partition); the weight matrices are
    pre-permuted on the host to match (matmul contracts lhsT and rhs by
    partition index, so any shared permutation is legal).
  - Gate bias is injected by a K=1 matmul (ones stationary) that also
    opens each PSUM accumulation group.
  - Elementwise gate math runs on ACT (sigmoid/tanh) and DVE in the
    (128, 192) group layout; lanes 32g+[16:32) carry junk that never
    crosses into valid lanes.

x is pre-transposed or the host into per-subchunk stationary layout so the
x_t^T k-tiles are contiguous DMA loads.
"""

import numpy as np
from contextlib import ExitStack

import concourse.bass as bass
import concourse.bacc as bacc
import concourse.tile as tile
from concourse import mybir
from concourse.bass_utils import run_bass_kernel_spmd

F32 = mybir.dt.float32

B, T, I, H, O = 64, 1024, 512, 768, 512
NC = 8
BL = B // 4          # 16 batch rows per core
HS = H // 4          # 192 H-dims per column group
NKH = H // 128       # 6 h k-tiles
NKX = I // 128       # 4 x k-tiles
S = 8                # time steps per sub-chunk (one DMA each)
XF = S * NKX * BL    # sbuf free size of one x sub-chunk (8*4*16 = 512)

_CACHE = {}


def _row_perm():
    """DD[m*128+p] = H-dim held by partition p of h^T k-tile m."""
    m = np.arange(NKH)[:, None]
    p = np.arange(128)[None, :]
    dd = 192 * (p // 32) + 32 * m + (p % 32)
    return dd.reshape(-1)  # (768,)


def _col_perm():
    """COLMAP[go*768 + c*384 + q] = row of the (4H, *) weight matrices.

    chunk A (c=0): q in [0,192) -> gate i, q in [192,384) -> gate g
    chunk B (c=1): q in [0,192) -> gate f, q in [192,384) -> gate o
    gate rows: f=[0:768), i=[768:1536), g=[1536:2304), o=[2304:3072).
    """
    cm = np.zeros(4 * H, dtype=np.int64)
    for go in range(4):
        for c in range(2):
            for half in range(2):
                gate = [[1, 2], [0, 3]][c][half]
                q0 = go * 768 + c * 384 + half * 192
                cm[q0:q0 + 192] = gate * H + 192 * go + np.arange(192)
    return cm


def _build(t_steps, s_steps):
    """Build the SPMD Bass module (same NEFF on all 8 cores)."""
    nsc = t_steps // (2 * s_steps) * 2   # number of sub-chunks (even)
    assert nsc * s_steps == t_steps
    xf = s_steps * NKX * BL
    niter = nsc // 2

    nc = bacc.Bacc("TRN2", target_bir_lowering=False, debug=False,
                   num_devices=NC)
    xdram = nc.dram_tensor("x_t", [(nsc + 2) * 128, xf], F32,
                           kind="ExternalInput").ap()
    whr = nc.dram_tensor("wh_r", [NKH * 128, 4 * H], F32,
                         kind="ExternalInput").ap()
    wxr = nc.dram_tensor("wx_r", [NKX * 128, 4 * H], F32,
                         kind="ExternalInput").ap()
    br = nc.dram_tensor("b_r", [1, 4 * H], F32, kind="ExternalInput").ap()
    wyr = nc.dram_tensor("wy_r", [NKH * 128, O], F32,
                         kind="ExternalInput").ap()
    h_out = nc.dram_tensor("h_out", [BL, H], F32, kind="ExternalOutput").ap()
    c_out = nc.dram_tensor("c_out", [BL, H], F32, kind="ExternalOutput").ap()
    y_out = nc.dram_tensor("y_out", [BL, O], F32, kind="ExternalOutput").ap()

    with tile.TileContext(nc) as tc, ExitStack() as ctx:
        const = ctx.enter_context(tc.tile_pool(name="const", bufs=1))
        state = ctx.enter_context(tc.tile_pool(name="state", bufs=1))
        ps = ctx.enter_context(tc.tile_pool(name="ps", bufs=1, space="PSUM"))

        # --- static weights ---
        wh_sb = []
        for m in range(NKH):
            t = const.tile([128, 4 * H], F32, tag=f"wh{m}")
            nc.sync.dma_start(t[:], whr[m * 128:(m + 1) * 128, :])
            wh_sb.append(t)
        wx_sb = []
        for k in range(NKX):
            t = const.tile([128, 4 * H], F32, tag=f"wx{k}")
            nc.sync.dma_start(t[:], wxr[k * 128:(k + 1) * 128, :])
            wx_sb.append(t)
        wy_sb = []
        for m in range(NKH):
            t = const.tile([128, O], F32, tag=f"wy{m}")
            nc.sync.dma_start(t[:], wyr[m * 128:(m + 1) * 128, :])
            wy_sb.append(t)
        b_sb = const.tile([1, 4 * H], F32, tag="b")
        nc.sync.dma_start(b_sb[:], br[:])
        ones_sb = const.tile([1, BL], F32, tag="ones")
        nc.gpsimd.memset(ones_sb[:], 1.0)

        # --- state (persist across loop iterations) ---
        h_grp = state.tile([128, HS], F32, tag="h_grp")
        c_grp = state.tile([128, HS], F32, tag="c_grp")
        hT = [state.tile([128, HS], F32, tag=f"hT{p}") for p in range(2)]
        xsb = [state.tile([128, xf], F32, tag=f"xsb{p}") for p in range(2)]
        nc.vector.memset(c_grp[:], 0.0)
        nc.vector.memset(hT[0][:], 0.0)

        # --- psum (double buffered by step parity) ---
        zA = [ps.tile([128, 2 * HS], F32, tag=f"zA{p}") for p in range(2)]
        zB = [ps.tile([128, 2 * HS], F32, tag=f"zB{p}") for p in range(2)]
        y_ps = ps.tile([128, O], F32, tag="y_ps")

        # --- gate sbuf tiles (double buffered by step parity) ---
        gA = [state.tile([128, 2 * HS], F32, tag=f"gA{p}") for p in range(2)]
        gB = [state.tile([128, 2 * HS], F32, tag=f"gB{p}") for p in range(2)]
        tct = [state.tile([128, HS], F32, tag=f"tct{p}") for p in range(2)]
        t2 = [state.tile([128, HS], F32, tag=f"t2{p}") for p in range(2)]

        def emit_step(par, xt, s_loc):
            """One LSTM time step. par = global step parity (0/1)."""
            h_in, h_nx = hT[par], hT[1 - par]
            za, zb = zA[par], zB[par]
            ga, gb, tc_t, tt2 = gA[par], gB[par], tct[par], t2[par]
            xof = s_loc * NKX * BL
            for cn, z in ((0, za), (1, zb)):
                for go in range(4):
                    zo = z[32 * go:32 * go + BL, :]
                    tp = (0, 32 * go)
                    cbase = go * 768 + cn * 384
                    nc.tensor.matmul(
                        zo, ones_sb[:, :], b_sb[:, cbase:cbase + 384],
                        start=True, stop=False, tile_position=tp)
                for k in range(NKX):
                    lx = xt[:, xof + k * BL: xof + (k + 1) * BL]
                    for go in range(4):
                        zo = z[32 * go:32 * go + BL, :]
                        cbase = go * 768 + cn * 384
                        nc.tensor.matmul(
                            zo, lx, wx_sb[k][:, cbase:cbase + 384],
                            start=False, stop=False,
                            tile_position=(0, 32 * go))
                for m in range(NKH):
                    lh = h_in[:, 32 * m: 32 * m + BL]
                    for go in range(4):
                        zo = z[32 * go:32 * go + BL, :]
                        cbase = go * 768 + cn * 384
                        nc.tensor.matmul(
                            zo, lh, wh_sb[m][:, cbase:cbase + 384],
                            start=False, stop=(m == NKH - 1),
                            tile_position=(0, 32 * go))
            # activations: A = [i | g], B = [f | o]
            nc.scalar.activation(ga[:, 0:HS], za[:, 0:HS],
                                 mybir.ActivationFunctionType.Sigmoid)
            nc.scalar.activation(ga[:, HS:2 * HS], za[:, HS:2 * HS],
                                 mybir.ActivationFunctionType.Tanh)
            nc.scalar.activation(gb[:, :], zb[:, :],
                                 mybir.ActivationFunctionType.Sigmoid)
            # c = f*c + i*g ; h = o*tanh(c)
            nc.vector.tensor_mul(tt2[:], ga[:, 0:HS], ga[:, HS:2 * HS])
            nc.vector.tensor_mul(c_grp[:], gb[:, 0:HS], c_grp[:])
            nc.vector.tensor_add(c_grp[:], c_grp[:], tt2[:])
            nc.scalar.activation(tc_t[:], c_grp[:],
                                 mybir.ActivationFunctionType.Tanh)
            nc.vector.tensor_mul(h_grp[:], gb[:, HS:2 * HS], tc_t[:])
            nc.vector.transpose(h_nx[:], h_grp[:])

        # prologue: load sub-chunk 0
        nc.sync.dma_start(xsb[0][:], xdram[0:128, :])

        par0 = (2 * s_steps) % 2  # = 0: every iteration starts at parity 0
        assert par0 == 0 and s_steps % 2 == 0

        with tc.For_i(0, niter, 1,
                      hint_engines=(mybir.EngineType.PE,
                                    mybir.EngineType.Activation,
                                    mybir.EngineType.DVE)) as it:
            row1 = it * 256 + 128
            nc.sync.dma_start(xsb[1][:], xdram[bass.ds(row1, 128), :])
            for s in range(s_steps):
                emit_step(s % 2, xsb[0], s)
            row2 = it * 256 + 256
            nc.sync.dma_start(xsb[0][:], xdram[bass.ds(row2, 128), :])
            for s in range(s_steps):
                emit_step(s % 2, xsb[1], s)

        # epilogue: final states out + partial y = h_final @ Why_slice.T
        for go in range(4):
            nc.sync.dma_start(h_out[:, 192 * go:192 * (go + 1)],
                              h_grp[32 * go:32 * go + BL, :])
            nc.sync.dma_start(c_out[:, 192 * go:192 * (go + 1)],
                              c_grp[32 * go:32 * go + BL, :])
        hT_fin = hT[t_steps % 2]
        for m in range(NKH):
            nc.tensor.matmul(y_ps[0:BL, :], hT_fin[:, 32 * m:32 * m + BL],
                             wy_sb[m][:, :], start=(m == 0),
                             stop=(m == NKH - 1))
        y_sb = state.tile([128, O], F32, tag="y_sb")
        nc.vector.tensor_copy(y_sb[0:BL, :], y_ps[0:BL, :])
        nc.sync.dma_start(y_out[:], y_sb[0:BL, :])

    nc.compile()
    return nc


def _get_module(t_steps=T, s_steps=S):
    key = (t_steps, s_steps)
    if key not in _CACHE:
        _CACHE[key] = _build(t_steps, s_steps)
    return _CACHE[key]


def _prep_core_inputs(core, x_seq, Wx_f, Wh_f, b_f, Wx_b, Wh_b, b_b, Why,
                      t_steps=T, s_steps=S):
    d, q = core // 4, core % 4
    Wx, Wh, bb = (Wx_f, Wh_f, b_f) if d == 0 else (Wx_b, Wh_b, b_b)
    nsc = t_steps // s_steps
    DD = _row_perm()
    CM = _col_perm()

    xq = np.asarray(x_seq[BL * q:BL * q + BL, :t_steps])   # (16, t, I)
    if d == 1:
        xq = xq[:, ::-1, :]
    # xarr[sc*128 + p, s*64 + k*16 + b] = xq[b, sc*S+s, 128k+p]
    xt = np.ascontiguousarray(xq.transpose(1, 2, 0))       # (t, I, 16)
    xt = xt.reshape(nsc, s_steps, NKX, 128, BL)
    xt = xt.transpose(0, 3, 1, 2, 4).reshape(nsc * 128, s_steps * NKX * BL)
    xarr = np.zeros(((nsc + 2) * 128, s_steps * NKX * BL), np.float32)
    xarr[:nsc * 128] = xt

    WhT = np.ascontiguousarray(Wh.T)       # (768, 3072)
    whr = np.ascontiguousarray(WhT[DD][:, CM])
    wxr = np.ascontiguousarray(Wx.T[:, CM])          # (512, 3072)
    brr = np.ascontiguousarray(bb[CM])[None, :]
    WyT = np.ascontiguousarray(Why.T)      # (1536, 512)
    wyr = np.ascontiguousarray(WyT[d * H:(d + 1) * H][DD])

    return {"x_t": xarr, "wh_r": whr, "wx_r": wxr, "b_r": brr, "wy_r": wyr}


def kernel(x_seq, Wx_f, Wh_f, b_f, Wx_b, Wh_b, b_b, Why, by):
    x_seq = np.asarray(x_seq, np.float32)
    args = (x_seq, np.asarray(Wx_f, np.float32), np.asarray(Wh_f, np.float32),
            np.asarray(b_f, np.float32), np.asarray(Wx_b, np.float32),
            np.asarray(Wh_b, np.float32), np.asarray(b_b, np.float32),
            np.asarray(Why, np.float32))
    nc = _get_module()
    in_maps = [_prep_core_inputs(c, *args) for c in range(NC)]
    res = run_bass_kernel_spmd(nc, in_maps, list(range(NC))).results
    h_f = np.concatenate([res[c]["h_out"] for c in range(4)], axis=0)
    c_f = np.concatenate([res[c]["c_out"] for c in range(4)], axis=0)
    h_b = np.concatenate([res[c]["h_out"] for c in range(4, 8)], axis=0)
    c_b = np.concatenate([res[c]["c_out"] for c in range(4, 8)], axis=0)
    out = np.concatenate(
        [res[c]["y_out"] + res[c + 4]["y_out"] for c in range(4)], axis=0)
    out = out + np.asarray(by, np.float32)[None, :]
    return out, h_f, c_f, h_b, c_b


# revision 2
# speedup vs baseline: 111.6867x; 111.6867x over previous
"""BiLSTM kernel for 8 Trainium2 NeuronCores.

Problem: B=64, T=1024, I=512, H=768, O=512 BiLSTM (gates f,i,g,o).

Sharding: core c in [0..7]: direction d = c//4 (0=fwd, 1=bwd over reversed
time), batch quarter q = c%4 (rows 16q..16q+16). Zero cross-core comms.

Per-core recurrence design (per time step, B_local=16):
  - z (16, 3072) = h @ Wh.T + x_t @ Wx.T + b computed as 4 concurrent
    column-group matmul streams (tile_position=(0, 32*g)): group g computes
    the H-slice [192g:192(g+1)) of every gate. PSUM chunk A = [i | g] gates,
    chunk B = [f | o] gates, each (128p, 384f) (partition window 32g+[0:16)
    = batch for group g).
  - Stationary operands are h^T tiles (128, 16) taken from hT, which is
    produced each step by a single DVE 32x32 block transpose of the
    group-layout h (128, 192). The block transpose leaves H-dims in a
    permuted order across (k-tile, partition); the weight matrices are
    pre-permuted on the host to match (matmul contracts lhsT and rhs by
    partition index, so any shared permutation is legal).
  - Gate bias is injected by a K=1 matmul (ones stationary) that also
    opens each PSUM accumulation group.
  - Elementwise gate math runs on ACT (sigmoid/tanh) and DVE in the
    (128, 192) group layout; lanes 32g+[16:32) carry junk that never
    crosses into valid lanes.

x is pre-transposed or the host into per-subchunk stationary layout so the
x_t^T k-tiles are contiguous DMA loads.
"""

import numpy as np
from contextlib import ExitStack

import concourse.bass as bass
import concourse.bacc as bacc
import concourse.tile as tile
from concourse import mybir
from concourse.bass_utils import run_bass_kernel_spmd

F32 = mybir.dt.float32

B, T, I, H, O = 64, 1024, 512, 768, 512
NC = 8
BL = B // 4          # 16 batch rows per core
HS = H // 4          # 192 H-dims per column group
NKH = H // 128       # 6 h k-tiles
NKX = I // 128       # 4 x k-tiles
S = 8                # time steps per sub-chunk (one DMA each)
XF = S * NKX * BL    # sbuf free size of one x sub-chunk (8*4*16 = 512)

_CACHE = {}


def _row_perm():
    """DD[m*128+p] = H-dim held by partition p of h^T k-tile m."""
    m = np.arange(NKH)[:, None]
    p = np.arange(128)[None, :]
    dd = 192 * (p // 32) + 32 * m + (p % 32)
    return dd.reshape(-1)  # (768,)


def _col_perm():
    """COLMAP[go*768 + c*384 + q] = row of the (4H, *) weight matrices.

    chunk A (c=0): q in [0,192) -> gate i, q in [192,384) -> gate g
    chunk B (c=1): q in [0,192) -> gate f, q in [192,384) -> gate o
    gate rows: f=[0:768), i=[768:1536), g=[1536:2304), o=[2304:3072).
    """
    cm = np.zeros(4 * H, dtype=np.int64)
    for go in range(4):
        for c in range(2):
            for half in range(2):
                gate = [[1, 2], [0, 3]][c][half]
                q0 = go * 768 + c * 384 + half * 192
                cm[q0:q0 + 192] = gate * H + 192 * go + np.arange(192)
    return cm


def _build(t_steps, s_steps):
    """Build the SPMD Bass module (same NEFF on all 8 cores)."""
    nsc = t_steps // (2 * s_steps) * 2   # number of sub-chunks (even)
    assert nsc * s_steps == t_steps
    xf = s_steps * NKX * BL
    niter = nsc // 2

    nc = bacc.Bacc("TRN2", target_bir_lowering=False, debug=False,
                   num_devices=NC)
    xdram = nc.dram_tensor("x_t", [(nsc + 2) * 128, xf], F32,
                           kind="ExternalInput").ap()
    whr = nc.dram_tensor("wh_r", [NKH * 128, 4 * H], F32,
                         kind="ExternalInput").ap()
    wxr = nc.dram_tensor("wx_r", [NKX * 128, 4 * H], F32,
                         kind="ExternalInput").ap()
    br = nc.dram_tensor("b_r", [1, 4 * H], F32, kind="ExternalInput").ap()
    wyr = nc.dram_tensor("wy_r", [NKH * 128, O], F32,
                         kind="ExternalInput").ap()
    h_out = nc.dram_tensor("h_out", [BL, H], F32, kind="ExternalOutput").ap()
    c_out = nc.dram_tensor("c_out", [BL, H], F32, kind="ExternalOutput").ap()
    y_out = nc.dram_tensor("y_out", [BL, O], F32, kind="ExternalOutput").ap()

    with tile.TileContext(nc) as tc, ExitStack() as ctx:
        const = ctx.enter_context(tc.tile_pool(name="const", bufs=1))
        state = ctx.enter_context(tc.tile_pool(name="state", bufs=1))
        ps = ctx.enter_context(tc.tile_pool(name="ps", bufs=1, space="PSUM"))

        # --- static weights ---
        wh_sb = []
        for m in range(NKH):
            t = const.tile([128, 4 * H], F32, tag=f"wh{m}", name=f"wh{m}")
            nc.sync.dma_start(t[:], whr[m * 128:(m + 1) * 128, :])
            wh_sb.append(t)
        wx_sb = []
        for k in range(NKX):
            t = const.tile([128, 4 * H], F32, tag=f"wx{k}", name=f"wx{k}")
            nc.sync.dma_start(t[:], wxr[k * 128:(k + 1) * 128, :])
            wx_sb.append(t)
        wy_sb = []
        for m in range(NKH):
            t = const.tile([128, O], F32, tag=f"wy{m}", name=f"wy{m}")
            nc.sync.dma_start(t[:], wyr[m * 128:(m + 1) * 128, :])
            wy_sb.append(t)
        b_sb = const.tile([1, 4 * H], F32, tag="b", name="b_t")
        nc.sync.dma_start(b_sb[:], br[:])
        ones_sb = const.tile([1, BL], F32, tag="ones", name="ones_t")
        nc.gpsimd.memset(ones_sb[:], 1.0)

        # --- state (persist across loop iterations) ---
        h_grp = state.tile([128, HS], F32, tag="h_grp", name="h_grp")
        c_grp = state.tile([128, HS], F32, tag="c_grp", name="c_grp")
        hT = [state.tile([128, HS], F32, tag=f"hT{p}", name=f"hT{p}") for p in range(2)]
        xsb = [state.tile([128, xf], F32, tag=f"xsb{p}", name=f"xsb{p}") for p in range(2)]
        nc.vector.memset(c_grp[:], 0.0)
        nc.vector.memset(hT[0][:], 0.0)

        # --- psum (double buffered by step parity) ---
        zA = [ps.tile([128, 2 * HS], F32, tag=f"zA{p}", name=f"zA{p}") for p in range(2)]
        zB = [ps.tile([128, 2 * HS], F32, tag=f"zB{p}", name=f"zB{p}") for p in range(2)]
        y_ps = ps.tile([128, O], F32, tag="y_ps", name="y_ps")

        # --- gate sbuf tiles (double buffered by step parity) ---
        gA = [state.tile([128, 2 * HS], F32, tag=f"gA{p}", name=f"gA{p}") for p in range(2)]
        gB = [state.tile([128, 2 * HS], F32, tag=f"gB{p}", name=f"gB{p}") for p in range(2)]
        tct = [state.tile([128, HS], F32, tag=f"tct{p}", name=f"tct{p}") for p in range(2)]
        t2 = [state.tile([128, HS], F32, tag=f"t2{p}", name=f"t2{p}") for p in range(2)]

        def emit_step(par, xt, s_loc):
            """One LSTM time step. par = global step parity (0/1)."""
            h_in, h_nx = hT[par], hT[1 - par]
            za, zb = zA[par], zB[par]
            ga, gb, tc_t, tt2 = gA[par], gB[par], tct[par], t2[par]
            xof = s_loc * NKX * BL
            for cn, z in ((0, za), (1, zb)):
                for go in range(4):
                    zo = z[32 * go:32 * go + BL, :]
                    tp = (0, 32 * go)
                    cbase = go * 768 + cn * 384
                    nc.tensor.matmul(
                        zo, ones_sb[:, :], b_sb[:, cbase:cbase + 384],
                        start=True, stop=False, tile_position=tp)
                for k in range(NKX):
                    lx = xt[:, xof + k * BL: xof + (k + 1) * BL]
                    for go in range(4):
                        zo = z[32 * go:32 * go + BL, :]
                        cbase = go * 768 + cn * 384
                        nc.tensor.matmul(
                            zo, lx, wx_sb[k][:, cbase:cbase + 384],
                            start=False, stop=False,
                            tile_position=(0, 32 * go))
                for m in range(NKH):
                    lh = h_in[:, 32 * m: 32 * m + BL]
                    for go in range(4):
                        zo = z[32 * go:32 * go + BL, :]
                        cbase = go * 768 + cn * 384
                        nc.tensor.matmul(
                            zo, lh, wh_sb[m][:, cbase:cbase + 384],
                            start=False, stop=(m == NKH - 1),
                            tile_position=(0, 32 * go))
            # activations: A = [i | g], B = [f | o]
            nc.scalar.activation(ga[:, 0:HS], za[:, 0:HS],
                                 mybir.ActivationFunctionType.Sigmoid)
            nc.scalar.activation(ga[:, HS:2 * HS], za[:, HS:2 * HS],
                                 mybir.ActivationFunctionType.Tanh)
            nc.scalar.activation(gb[:, :], zb[:, :],
                                 mybir.ActivationFunctionType.Sigmoid)
            # c = f*c + i*g ; h = o*tanh(c)
            nc.vector.tensor_mul(tt2[:], ga[:, 0:HS], ga[:, HS:2 * HS])
            nc.vector.tensor_mul(c_grp[:], gb[:, 0:HS], c_grp[:])
            nc.vector.tensor_add(c_grp[:], c_grp[:], tt2[:])
            nc.scalar.activation(tc_t[:], c_grp[:],
                                 mybir.ActivationFunctionType.Tanh)
            nc.vector.tensor_mul(h_grp[:], gb[:, HS:2 * HS], tc_t[:])
            nc.vector.transpose(h_nx[:], h_grp[:])

        # prologue: load sub-chunk 0
        nc.sync.dma_start(xsb[0][:], xdram[0:128, :])

        par0 = (2 * s_steps) % 2  # = 0: every iteration starts at parity 0
        assert par0 == 0 and s_steps % 2 == 0

        with tc.For_i(0, niter, 1,
                      hint_engines=(mybir.EngineType.PE,
                                    mybir.EngineType.Activation,
                                    mybir.EngineType.DVE)) as it:
            row1 = it * 256 + 128
            nc.sync.dma_start(xsb[1][:], xdram[bass.ds(row1, 128), :])
            for s in range(s_steps):
                emit_step(s % 2, xsb[0], s)
            row2 = it * 256 + 256
            nc.sync.dma_start(xsb[0][:], xdram[bass.ds(row2, 128), :])
            for s in range(s_steps):
                emit_step(s % 2, xsb[1], s)

        # epilogue: final states out + partial y = h_final @ Why_slice.T
        for go in range(4):
            nc.sync.dma_start(h_out[:, 192 * go:192 * (go + 1)],
                              h_grp[32 * go:32 * go + BL, :])
            nc.sync.dma_start(c_out[:, 192 * go:192 * (go + 1)],
                              c_grp[32 * go:32 * go + BL, :])
        hT_fin = hT[t_steps % 2]
        for m in range(NKH):
            nc.tensor.matmul(y_ps[0:BL, :], hT_fin[:, 32 * m:32 * m + BL],
                             wy_sb[m][:, :], start=(m == 0),
                             stop=(m == NKH - 1))
        y_sb = state.tile([128, O], F32, tag="y_sb", name="y_sb")
        nc.vector.tensor_copy(y_sb[0:BL, :], y_ps[0:BL, :])
        nc.sync.dma_start(y_out[:], y_sb[0:BL, :])

    nc.compile()
    return nc


def _get_module(t_steps=T, s_steps=S):
    key = (t_steps, s_steps)
    if key not in _CACHE:
        _CACHE[key] = _build(t_steps, s_steps)
    return _CACHE[key]


def _prep_core_inputs(core, x_seq, Wx_f, Wh_f, b_f, Wx_b, Wh_b, b_b, Why,
                      t_steps=T, s_steps=S):
    d, q = core // 4, core % 4
    Wx, Wh, bb = (Wx_f, Wh_f, b_f) if d == 0 else (Wx_b, Wh_b, b_b)
    nsc = t_steps // s_steps
    DD = _row_perm()
    CM = _col_perm()

    xq = np.asarray(x_seq[BL * q:BL * q + BL, :t_steps])   # (16, t, I)
    if d == 1:
        xq = xq[:, ::-1, :]
    # xarr[sc*128 + p, s*64 + k*16 + b] = xq[b, sc*S+s, 128k+p]
    xt = np.ascontiguousarray(xq.transpose(1, 2, 0))       # (t, I, 16)
    xt = xt.reshape(nsc, s_steps, NKX, 128, BL)
    xt = xt.transpose(0, 3, 1, 2, 4).reshape(nsc * 128, s_steps * NKX * BL)
    xarr = np.zeros(((nsc + 2) * 128, s_steps * NKX * BL), np.float32)
    xarr[:nsc * 128] = xt

    WhT = np.ascontiguousarray(Wh.T)       # (768, 3072)
    whr = np.ascontiguousarray(WhT[DD][:, CM])
    wxr = np.ascontiguousarray(Wx.T[:, CM])          # (512, 3072)
    brr = np.ascontiguousarray(bb[CM])[None, :]
    WyT = np.ascontiguousarray(Why.T)      # (1536, 512)
    wyr = np.ascontiguousarray(WyT[d * H:(d + 1) * H][DD])

    return {"x_t": xarr, "wh_r": whr, "wx_r": wxr, "b_r": brr, "wy_r": wyr}


def kernel(x_seq, Wx_f, Wh_f, b_f, Wx_b, Wh_b, b_b, Why, by):
    x_seq = np.asarray(x_seq, np.float32)
    args = (x_seq, np.asarray(Wx_f, np.float32), np.asarray(Wh_f, np.float32),
            np.asarray(b_f, np.float32), np.asarray(Wx_b, np.float32),
            np.asarray(Wh_b, np.float32), np.asarray(b_b, np.float32),
            np.asarray(Why, np.float32))
    nc = _get_module()
    in_maps = [_prep_core_inputs(c, *args) for c in range(NC)]
    res = run_bass_kernel_spmd(nc, in_maps, list(range(NC))).results
    h_f = np.concatenate([res[c]["h_out"] for c in range(4)], axis=0)
    c_f = np.concatenate([res[c]["c_out"] for c in range(4)], axis=0)
    h_b = np.concatenate([res[c]["h_out"] for c in range(4, 8)], axis=0)
    c_b = np.concatenate([res[c]["c_out"] for c in range(4, 8)], axis=0)
    out = np.concatenate(
        [res[c]["y_out"] + res[c + 4]["y_out"] for c in range(4)], axis=0)
    out = out + np.asarray(by, np.float32)[None, :]
    return out, h_f, c_f, h_b, c_b
